# revision 12
# baseline (speedup 1.0000x reference)
"""Bass/Trainium2 kernel for nn_ExaoneMoEAttention (sliding-window GQA attention).

Strategy (8 NeuronCores, tensor-parallel over heads):
  - core c owns q heads 4c..4c+3 and kv head c (w_qkv column shard [4096, 768]),
    plus w_o rows 512c..512c+512 ([512, 4096]).
  - hidden replicated, host-transposed/blocked; fp32r/bf16 matmuls.
  - Phase A (QKV proj): per 128-row t-tile, hidT tiles are the stationary
    operand and w_qkv columns the moving operand (qkv lands in [t, c] psum);
    RMSNorm stats via ACT Square+accum_out on the free dim; the normalized
    q/k head tiles are PE-transposed to [d, t] strips resident in SBUF, with
    norm-weight (and softmax scale for q) folded into the transpose
    evacuation; RoPE via host cos/sin tables; v needs no transpose.
  - Phase B: 256-row q sub-slabs; per key tile one [128, 512] psum holds both
    heads of a pair (scores), one Exp covers the pair; sliding window (1024)
    + causal via multiplicative 0/1 bf16 masks on DVE (only 4 edge deltas);
    softmax without max-subtraction (RMSNorm bounds |score| <= sqrt(D));
    exp-sums via a 128-wide ones stationary (the sum lands broadcast across
    all psum partitions, so normalization is a straight DVE reciprocal+mul,
    no (219+N) M=1 matmul and no broadcast matmul); GQA lets one AV matmul
    serve both heads of the pair (shared kv head).
  - o_proj per 256-row sub-slab (interleaved into the next sub-slab's
    attention), then bf16 ReduceScatter(add) per sub-slab so only the last
    ~27us collective is exposed; host concatenates the 8 row-shards.
"""

import ml_dtypes
import numpy as np

import concourse.bass as bass
import concourse.mybir as mybir
import concourse.tile as tile
from concourse import bacc
from concourse.bass_utils import run_bass_kernel_spmd
from concourse.masks import make_identity

F32 = mybir.dt.float32
F32R = mybir.dt.float32r
BF16 = mybir.dt.bfloat16
AF = mybir.ActivationFunctionType

N_CORES = 8
T = 2048
HID = 4096
H = 32
HKV = 8
D = 128
EPS = 1e-5
THETA = 1e6
WINDOW = 1024

HL = H // N_CORES          # 4 local q heads
NT = T // 128              # 16 t/s tiles
KO = HID // 128            # 32 k-subtiles in projection
SC = 256                   # q sub-slab in attention phase
N_SC = T // SC             # 8
ECH = 512                  # o_proj e-chunk
N_ECH = HID // ECH         # 8
# reduce-scatter row groups: coarse early (amortize the ~13us fixed cost per
# collective), small at the end (minimize the exposed tail)
RS_GROUPS = [(0, 512), (512, 1024), (1024, 1536), (1536, 1792), (1792, 2048)]
RS_AFTER_SC = {(r1 - 1) // SC: g for g, (r0, r1) in enumerate(RS_GROUPS)}

MASK_DELTAS = [0, -128, 896, 1024]
MASK_IDX = {d: i for i, d in enumerate(MASK_DELTAS)}


def _build():
    nc = bacc.Bacc(num_devices=N_CORES)

    # hidT4[ki, tt, ko, j] = hidden[tt*128+j, ko*128+ki]
    hidT = nc.declare_dram_parameter("hidT", [128, NT, KO, 128], BF16, isOutput=False)
    wq = nc.declare_dram_parameter("wq", [128, KO, (HL + 2) * D], BF16, isOutput=False)
    wo = nc.declare_dram_parameter("wo", [128, HL, HID], BF16, isOutput=False)
    cs2 = nc.declare_dram_parameter("cs2", [128, T], F32, isOutput=False)
    sn2s = nc.declare_dram_parameter("sn2s", [128, T], F32, isOutput=False)
    masks = nc.declare_dram_parameter("masks", [128, len(MASK_DELTAS), 2 * SC], BF16, isOutput=False)
    qwv = nc.declare_dram_parameter("qwv", [D, 1], F32, isOutput=False)
    kwv = nc.declare_dram_parameter("kwv", [D, 1], F32, isOutput=False)
    onc_d = nc.declare_dram_parameter("onc", [128, 128], BF16, isOutput=False)
    out_p = nc.declare_dram_parameter("out", [T // N_CORES, HID], BF16, isOutput=True)

    with tile.TileContext(nc) as tc:
        with tc.tile_pool(name="persistA", bufs=1) as pA:
            kT = pA.tile([128, T], F32R)                     # rope'd k, [d, s]
            qT = [pA.tile([128, T], F32R, name=f"qT{h}") for h in range(HL)]
            vnat = pA.tile([128, NT, D], BF16)               # v in [s, d] tiles
            onc = pA.tile([128, 128], BF16)
            ident = pA.tile([128, 128], BF16)
            make_identity(nc, ident[:])
            nc.sync.dma_start(out=onc[:], in_=onc_d[:])

            # ---------------- Phase A: QKV projection + norm + rope ----------
            with (
                tc.tile_pool(name="wpool", bufs=1) as wpool,
                tc.tile_pool(name="hidp", bufs=3) as hidp,
                tc.tile_pool(name="cspool", bufs=2) as cspool,
                tc.tile_pool(name="tmpA", bufs=6) as tmpA,
                tc.tile_pool(name="stA", bufs=6) as stA,
                tc.tile_pool(name="miscA", bufs=1) as miscA,
                tc.tile_pool(name="psq", bufs=3, space="PSUM") as psq_p,
                tc.tile_pool(name="psvt", bufs=2, space="PSUM") as psvt_p,
            ):
                # wq in 8 chunks of 4 ko so the first matmuls gate on ~0.8MB,
                # issue-interleaved with the first hid tiles
                NWG = 8
                KOG = KO // NWG
                w_grp = [
                    wpool.tile([128, KOG, (HL + 2) * D], BF16, name=f"w{g}")
                    for g in range(NWG)
                ]
                nc.sync.dma_start(out=w_grp[0][:], in_=wq[:, 0:KOG, :])
                qw_sb = miscA.tile([D, 1], F32)
                kw_sb = miscA.tile([D, 1], F32)
                eps_sb = miscA.tile([128, 1], F32)
                nc.sync.dma_start(out=qw_sb[:], in_=qwv[:])
                nc.sync.dma_start(out=kw_sb[:], in_=kwv[:])
                nc.vector.memset(eps_sb[:], EPS)

                pending_post = []

                def flush_post(keep=0):
                    while len(pending_post) > keep:
                        pending_post.pop(0)()

                for tt in range(NT):
                    tsl = slice(tt * 128, (tt + 1) * 128)
                    hid_t = hidp.tile([128, KO, 128], BF16, tag="hid")
                    nc.sync.dma_start(out=hid_t[:], in_=hidT[:, tt])
                    if tt == 0:
                        for g in range(1, NWG):
                            nc.sync.dma_start(
                                out=w_grp[g][:], in_=wq[:, g * KOG:(g + 1) * KOG, :]
                            )
                    cs_t = cspool.tile([128, 128], F32, tag="cs")
                    sn_t = cspool.tile([128, 128], F32, tag="sn")
                    nc.sync.dma_start(out=cs_t[:], in_=cs2[:, tsl])
                    nc.sync.dma_start(out=sn_t[:], in_=sn2s[:, tsl])

                    # qkv[t, c] for this t-tile: [128, 512] + [128, 256] psums
                    pq = psq_p.tile([128, 4 * D], F32, tag="pq")
                    pq2 = psq_p.tile([128, 2 * D], F32, tag="pq2")
                    for ko in range(KO):
                        lhsT = hid_t[:, ko, :]
                        wg = w_grp[ko // KOG]
                        nc.tensor.matmul(
                            pq[:], lhsT, wg[:, ko % KOG, 0:4 * D],
                            start=(ko == 0), stop=(ko == KO - 1),
                        )
                        nc.tensor.matmul(
                            pq2[:], lhsT, wg[:, ko % KOG, 4 * D:6 * D],
                            start=(ko == 0), stop=(ko == KO - 1),
                        )
                    flush_post(keep=1)

                    def make_post(tt=tt, pq=pq, pq2=pq2, tsl=tsl, cs_t=cs_t, sn_t=sn_t):
                        def _post():
                            # stage-parallel across the 5 normed heads so the
                            # ACT/DVE chains pipeline instead of serializing
                            srcs = [pq[:, m * D:(m + 1) * D] for m in range(HL)] + [pq2[:, 0:D]]
                            var, sd, rstd, ev, tp, qd, qsw = [], [], [], [], [], [], []
                            for m in range(HL + 1):
                                sqd = tmpA.tile([128, D], F32, tag="sqd", name="sqd")
                                var.append(stA.tile([128, 1], F32, tag="var", name="var"))
                                nc.scalar.activation(sqd[:], srcs[m], AF.Square, accum_out=var[m][:])
                            for m in range(HL + 1):
                                sd.append(stA.tile([128, 1], F32, tag="sd", name="sd"))
                                nc.scalar.activation(sd[m][:], var[m][:], AF.Sqrt, scale=1.0 / D, bias=eps_sb[:])
                            for m in range(HL + 1):
                                rstd.append(stA.tile([128, 1], F32, tag="rstd", name="rstd"))
                                nc.vector.reciprocal(rstd[m][:], sd[m][:])
                            for m in range(HL + 1):
                                ev.append(tmpA.tile([128, D], BF16, tag="ev", name="ev"))
                                nc.scalar.activation(ev[m][:], srcs[m], AF.Copy, scale=rstd[m][:])
                            for m in range(HL + 1):
                                tp.append(psvt_p.tile([128, 128], BF16, tag="tp", name="tp"))
                                nc.tensor.transpose(tp[m][:], ev[m][:], ident[:])
                            for m in range(HL + 1):
                                qd.append(tmpA.tile([128, D], F32, tag="qd", name="qd"))
                                nc.scalar.activation(
                                    qd[m][:], tp[m][:], AF.Copy,
                                    scale=(qw_sb[:] if m < HL else kw_sb[:]),
                                )
                            for m in range(HL + 1):
                                qsw.append(tmpA.tile([128, D], F32, tag="qsw", name="qsw"))
                                nc.vector.tensor_copy(qsw[m][0:64, :], qd[m][64:128, :])
                                nc.vector.tensor_copy(qsw[m][64:128, :], qd[m][0:64, :])
                            for m in range(HL + 1):
                                nc.vector.tensor_mul(qd[m][:], qd[m][:], cs_t[:])
                                nc.vector.tensor_mul(qsw[m][:], qsw[m][:], sn_t[:])
                                dst = qT[m][:, tsl] if m < HL else kT[:, tsl]
                                nc.vector.tensor_add(dst, qd[m][:], qsw[m][:])
                            nc.vector.tensor_copy(vnat[:, tt, :], pq2[:, D:2 * D])
                        return _post

                    pending_post.append(make_post())
                flush_post()

            # ---------------- Phase B: attention + o_proj + reduce-scatter ---
            with (
                tc.tile_pool(name="persistB", bufs=1) as pB,
                tc.tile_pool(name="exp", bufs=6) as exp_p,
                tc.tile_pool(name="stB", bufs=2) as stB,
                tc.tile_pool(name="ostg", bufs=8) as ostg_p,
                tc.tile_pool(name="psc", bufs=4, space="PSUM") as psc_p,
                tc.tile_pool(name="psav", bufs=1, space="PSUM") as psav_p,
                tc.tile_pool(name="pssum", bufs=1, space="PSUM") as pssum_p,
                tc.tile_pool(name="pso", bufs=2, space="PSUM") as pso_p,
                tc.tile_pool(name="dramB", bufs=1, space="DRAM") as dramB,
            ):
                attnT = pB.tile([128, HL, T], BF16)
                wo_sb = pB.tile([128, HL, HID], BF16)
                mask_sb = pB.tile([128, len(MASK_DELTAS), 2 * SC], BF16)
                nc.sync.dma_start(out=mask_sb[:], in_=masks[:])
                nc.gpsimd.dma_start(out=wo_sb[:], in_=wo[:])

                # one partial tile per RS group: a shared tile would make every
                # o_proj DMA write for later rows depend on the in-flight RS
                # read of earlier rows (observed as 10us PE stalls)
                partial = [
                    dramB.tile([r1 - r0, HID], BF16, name=f"partial{g}")
                    for g, (r0, r1) in enumerate(RS_GROUPS)
                ]
                rs_out = [
                    dramB.tile([(r1 - r0) // N_CORES, HID], BF16, name=f"rsout{g}")
                    for g, (r0, r1) in enumerate(RS_GROUPS)
                ]

                def rs_group_of(trow):
                    for g, (r0, r1) in enumerate(RS_GROUPS):
                        if r0 <= trow * 128 < r1:
                            return g, r0
                    raise AssertionError(trow)

                # o_proj emitted as small PE groups, interleaved into the next
                # sub-slab's attention loop as filler so the PE stream stays
                # dense while the norm chain drains on DVE
                oproj_q = []

                def emit_oproj_group():
                    if oproj_q:
                        oproj_q.pop(0)()

                def queue_oproj(sc):
                    def make_group(trow, ec, rs_group):
                        def _g():
                            pso = pso_p.tile([128, ECH], F32, tag="pso", name="pso")
                            for h in range(HL):
                                nc.tensor.matmul(
                                    pso[:],
                                    attnT[:, h, trow * 128:(trow + 1) * 128],
                                    wo_sb[:, h, ec * ECH:(ec + 1) * ECH],
                                    start=(h == 0),
                                    stop=(h == HL - 1),
                                )
                            ost = ostg_p.tile([128, ECH], BF16, tag="ost", name="ost")
                            nc.scalar.activation(ost[:], pso[:], AF.Copy)
                            g, gr0 = rs_group_of(trow)
                            lrow = trow * 128 - gr0
                            nc.sync.dma_start(
                                out=partial[g][lrow:lrow + 128,
                                               ec * ECH:(ec + 1) * ECH],
                                in_=ost[:],
                            )
                            if rs_group is not None:
                                r0, r1 = RS_GROUPS[rs_group]
                                rows_c = (r1 - r0) // N_CORES
                                nc.gpsimd.collective_compute(
                                    "ReduceScatter",
                                    mybir.AluOpType.add,
                                    replica_groups=[list(range(N_CORES))],
                                    ins=[partial[rs_group][:]],
                                    outs=[rs_out[rs_group][:]],
                                )
                                # same gpsimd queue as the RS, so this wait
                                # can't poison other engines
                                o0 = r0 // N_CORES
                                nc.gpsimd.dma_start(
                                    out=out_p[o0:o0 + rows_c, :],
                                    in_=rs_out[rs_group][:],
                                )
                        return _g

                    rows = [sc * 2, sc * 2 + 1]
                    n = len(rows) * N_ECH
                    i = 0
                    for trow in rows:
                        for ec in range(N_ECH):
                            i += 1
                            rsg = RS_AFTER_SC.get(sc) if i == n else None
                            oproj_q.append(make_group(trow, ec, rsg))

                # flattened (sc, hp, si) stream with lookahead across (sc, hp)
                # boundaries so the PE never drains on the exp/norm chains
                steps = []
                for sc in range(N_SC):
                    q0 = sc * SC
                    si_lo = max(0, (q0 - (WINDOW - 1)) // 128)
                    sis = list(range(si_lo, 2 * sc + 2))
                    for hp in range(0, HL, 2):
                        for si in sis:
                            steps.append((sc, hp, si, sis[0], sis[-1]))

                exs = {}
                acc = {}

                def emit_scores(step):
                    sc, hp, si, _, _ = step
                    q0 = sc * SC
                    qsl = slice(q0, q0 + SC)
                    delta = q0 - si * 128
                    # both heads of the pair share one psum/exp tile
                    # ([:, 0:SC] = head hp, [:, SC:2SC] = head hp+1)
                    psc = psc_p.tile([128, 2 * SC], F32, tag="sc")
                    for j in range(2):
                        nc.tensor.matmul(
                            psc[:, j * SC:(j + 1) * SC],
                            kT[:, si * 128:(si + 1) * 128],
                            qT[hp + j][:, qsl], start=True, stop=True,
                        )
                    ex = exp_p.tile([128, 2 * SC], BF16, tag="ex")
                    nc.scalar.activation(ex[:], psc[:], AF.Exp)
                    if delta in MASK_IDX:
                        nc.vector.tensor_mul(
                            ex[:], ex[:], mask_sb[:, MASK_IDX[delta], :]
                        )
                    exs[(sc, hp, si)] = ex

                def emit_consume(step):
                    sc, hp, si, si0, si_last = step
                    q0 = sc * SC
                    qsl = slice(q0, q0 + SC)
                    first = si == si0
                    last = si == si_last
                    if first:
                        # GQA: one AV / one sum matmul serves both heads
                        acc[(sc, hp)] = (
                            psav_p.tile([128, 2 * SC], F32, tag="av", name="av"),
                            pssum_p.tile([128, 2 * SC], F32, tag="sum", name="sum"),
                        )
                    avs2, sums2 = acc[(sc, hp)]
                    ex = exs.pop((sc, hp, si))
                    nc.tensor.matmul(sums2[:], onc[:], ex[:], start=first, stop=last)
                    nc.tensor.matmul(avs2[:], vnat[:, si, :], ex[:], start=first, stop=last)
                    if last:
                        avs2, sums2 = acc.pop((sc, hp))
                        rws2 = stB.tile([128, 2 * SC], F32, tag="rws", name="rws")
                        nc.vector.reciprocal(rws2[:], sums2[:])
                        for j in range(2):
                            nc.vector.tensor_mul(
                                attnT[:, hp + j, qsl],
                                avs2[:, j * SC:(j + 1) * SC],
                                rws2[:, j * SC:(j + 1) * SC],
                            )
                        if hp == 2:
                            queue_oproj(sc)

                LOOK = 2
                for idx, step in enumerate(steps):
                    emit_scores(step)
                    if idx >= LOOK:
                        emit_consume(steps[idx - LOOK])
                        emit_oproj_group()
                        emit_oproj_group()
                for idx in range(len(steps) - LOOK, len(steps)):
                    emit_consume(steps[idx])
                    emit_oproj_group()
                    emit_oproj_group()
                while oproj_q:
                    emit_oproj_group()

    nc.finalize()
    return nc


_NC_CACHE = None


def _get_nc():
    global _NC_CACHE
    if _NC_CACHE is None:
        _NC_CACHE = _build()
    return _NC_CACHE


def _host_inputs(positions, hidden_states, w_qkv, q_norm_w, k_norm_w, w_o):
    positions = np.asarray(positions)
    hidden_states = np.asarray(hidden_states, dtype=np.float32)
    w_qkv = np.asarray(w_qkv, dtype=np.float32)
    q_norm_w = np.asarray(q_norm_w, dtype=np.float32)
    k_norm_w = np.asarray(k_norm_w, dtype=np.float32)
    w_o = np.asarray(w_o, dtype=np.float32)

    # [ki, tt, ko, j]
    hidT4 = np.ascontiguousarray(
        hidden_states.T.reshape(KO, 128, NT, 128).transpose(1, 2, 0, 3)
    ).astype(ml_dtypes.bfloat16)

    half = D // 2
    inv_freq = 1.0 / (THETA ** (np.arange(half, dtype=np.float32) / half))
    ang = positions.astype(np.float32)[:, None] * inv_freq[None, :]  # [T, 64]
    cos = np.cos(ang).T.astype(np.float32)   # [64, T]
    sin = np.sin(ang).T.astype(np.float32)
    cs2 = np.concatenate([cos, cos], axis=0)          # [128, T]
    sn2s = np.concatenate([-sin, sin], axis=0)        # [128, T]

    # multiplicative 0/1 masks on the exp output, duplicated for the two
    # heads sharing one [128, 2*SC] tile
    mk = np.zeros((len(MASK_DELTAS), 128, SC), np.float32)
    ss = np.arange(128)[:, None]
    ttv = np.arange(SC)[None, :]
    for i, dlt in enumerate(MASK_DELTAS):
        diff = dlt + ttv - ss
        mk[i] = ((diff >= 0) & (diff < WINDOW)).astype(np.float32)
    mk = np.concatenate([mk, mk], axis=2)  # [nd, 128, 2*SC]
    mk = np.ascontiguousarray(mk.transpose(1, 0, 2)).astype(ml_dtypes.bfloat16)

    qwv = (q_norm_w * (D ** -0.5)).reshape(D, 1).astype(np.float32)
    kwv = k_norm_w.reshape(D, 1).astype(np.float32)
    onc = np.ones((128, 128), ml_dtypes.bfloat16)

    in_maps = []
    for c in range(N_CORES):
        wq_c = np.concatenate(
            [
                w_qkv[:, c * HL * D:(c + 1) * HL * D],
                w_qkv[:, H * D + c * D:H * D + (c + 1) * D],
                w_qkv[:, (H + HKV) * D + c * D:(H + HKV) * D + (c + 1) * D],
            ],
            axis=1,
        )
        wq_c = np.ascontiguousarray(wq_c.reshape(KO, 128, (HL + 2) * D).transpose(1, 0, 2)).astype(ml_dtypes.bfloat16)
        wo_c = np.ascontiguousarray(
            w_o[c * HL * D:(c + 1) * HL * D, :].reshape(HL, 128, HID).transpose(1, 0, 2)
        ).astype(ml_dtypes.bfloat16)
        in_maps.append(
            {
                "hidT": hidT4,
                "wq": wq_c,
                "wo": wo_c,
                "cs2": cs2,
                "sn2s": sn2s,
                "masks": mk,
                "qwv": qwv,
                "kwv": kwv,
                "onc": onc,
            }
        )
    return in_maps


def _assemble(results):
    out = np.empty((T, HID), np.float32)
    for c in range(N_CORES):
        r = np.asarray(results[c]["out"], dtype=np.float32)  # [T//8, HID]
        for r0, r1 in RS_GROUPS:
            rows_c = (r1 - r0) // N_CORES
            o0 = r0 // N_CORES
            out[r0 + c * rows_c: r0 + (c + 1) * rows_c] = r[o0:o0 + rows_c]
    return out


def run_spmd(in_maps, trace=False, **kw):
    nc = _get_nc()
    return run_bass_kernel_spmd(nc, in_maps, list(range(N_CORES)), trace=trace, **kw)


def kernel(positions, hidden_states, w_qkv, q_norm_w, k_norm_w, w_o):
    in_maps = _host_inputs(positions, hidden_states, w_qkv, q_norm_w, k_norm_w, w_o)
    last_err = None
    for _ in range(3):
        try:
            res = run_spmd(in_maps)
            return _assemble(res.results)
        except Exception as e:  # rare transient NRT_EXEC_UNIT_UNRECOVERABLE
            last_err = e
    raise last_err


# revision 13
# speedup vs baseline: 1.0146x; 1.0146x over previous
"""Bass/Trainium2 kernel for nn_ExaoneMoEAttention (sliding-window GQA attention).

Strategy (8 NeuronCores, tensor-parallel over heads):
  - core c owns q heads 4c..4c+3 and kv head c (w_qkv column shard [4096, 768]),
    plus w_o rows 512c..512c+512 ([512, 4096]).
  - hidden replicated, host-transposed/blocked; fp32r/bf16 matmuls.
  - Phase A (QKV proj): per 128-row t-tile, hidT tiles are the stationary
    operand and w_qkv columns the moving operand (qkv lands in [t, c] psum);
    RMSNorm stats via ACT Square+accum_out on the free dim; the normalized
    q/k head tiles are PE-transposed to [d, t] strips resident in SBUF, with
    norm-weight (and softmax scale for q) folded into the transpose
    evacuation; RoPE via host cos/sin tables; v needs no transpose.
  - Phase B: 256-row q sub-slabs; per key tile one [128, 512] psum holds both
    heads of a pair (scores), one Exp covers the pair; sliding window (1024)
    + causal via multiplicative 0/1 bf16 masks on DVE (only 4 edge deltas);
    softmax without max-subtraction (RMSNorm bounds |score| <= sqrt(D));
    exp-sums via a 128-wide ones stationary (the sum lands broadcast across
    all psum partitions, so normalization is a straight DVE reciprocal+mul,
    no (219+N) M=1 matmul and no broadcast matmul); GQA lets one AV matmul
    serve both heads of the pair (shared kv head).
  - o_proj per 256-row sub-slab (interleaved into the next sub-slab's
    attention), then bf16 ReduceScatter(add) per sub-slab so only the last
    ~27us collective is exposed; host concatenates the 8 row-shards.
"""

import ml_dtypes
import numpy as np

import concourse.bass as bass
import concourse.mybir as mybir
import concourse.tile as tile
from concourse import bacc
from concourse.bass_utils import run_bass_kernel_spmd
from concourse.masks import make_identity

F32 = mybir.dt.float32
F32R = mybir.dt.float32r
BF16 = mybir.dt.bfloat16
AF = mybir.ActivationFunctionType

N_CORES = 8
T = 2048
HID = 4096
H = 32
HKV = 8
D = 128
EPS = 1e-5
THETA = 1e6
WINDOW = 1024

HL = H // N_CORES          # 4 local q heads
NT = T // 128              # 16 t/s tiles
KO = HID // 128            # 32 k-subtiles in projection
SC = 256                   # q sub-slab in attention phase
N_SC = T // SC             # 8
ECH = 512                  # o_proj e-chunk
N_ECH = HID // ECH         # 8
# reduce-scatter row groups: coarse early (amortize the ~13us fixed cost per
# collective), small at the end (minimize the exposed tail)
RS_GROUPS = [(0, 512), (512, 1024), (1024, 1536), (1536, 1792), (1792, 2048)]
RS_AFTER_SC = {(r1 - 1) // SC: g for g, (r0, r1) in enumerate(RS_GROUPS)}

MASK_DELTAS = [0, -128, 896, 1024]
MASK_IDX = {d: i for i, d in enumerate(MASK_DELTAS)}


def _build():
    nc = bacc.Bacc(num_devices=N_CORES)

    # hidT4[ki, tt, ko, j] = hidden[tt*128+j, ko*128+ki]
    hidT = nc.declare_dram_parameter("hidT", [128, NT, KO, 128], BF16, isOutput=False)
    wq = nc.declare_dram_parameter("wq", [128, KO, (HL + 2) * D], BF16, isOutput=False)
    wo = nc.declare_dram_parameter("wo", [128, HL, HID], BF16, isOutput=False)
    cs2 = nc.declare_dram_parameter("cs2", [128, T], F32, isOutput=False)
    sn2s = nc.declare_dram_parameter("sn2s", [128, T], F32, isOutput=False)
    masks = nc.declare_dram_parameter("masks", [128, len(MASK_DELTAS), 2 * SC], BF16, isOutput=False)
    qwv = nc.declare_dram_parameter("qwv", [D, 1], F32, isOutput=False)
    kwv = nc.declare_dram_parameter("kwv", [D, 1], F32, isOutput=False)
    onc_d = nc.declare_dram_parameter("onc", [128, 128], BF16, isOutput=False)
    out_p = nc.declare_dram_parameter("out", [T // N_CORES, HID], BF16, isOutput=True)

    with tile.TileContext(nc) as tc:
        with tc.tile_pool(name="persistA", bufs=1) as pA:
            kT = pA.tile([128, T], F32R)                     # rope'd k, [d, s]
            qT = [pA.tile([128, T], F32R, name=f"qT{h}") for h in range(HL)]
            vnat = pA.tile([128, NT, D], BF16)               # v in [s, d] tiles
            onc = pA.tile([128, 128], BF16)
            ident = pA.tile([128, 128], BF16)
            make_identity(nc, ident[:])
            nc.sync.dma_start(out=onc[:], in_=onc_d[:])

            # ---------------- Phase A: QKV projection + norm + rope ----------
            with (
                tc.tile_pool(name="wpool", bufs=1) as wpool,
                tc.tile_pool(name="hidp", bufs=3) as hidp,
                tc.tile_pool(name="cspool", bufs=2) as cspool,
                tc.tile_pool(name="tmpA", bufs=6) as tmpA,
                tc.tile_pool(name="stA", bufs=6) as stA,
                tc.tile_pool(name="miscA", bufs=1) as miscA,
                tc.tile_pool(name="psq", bufs=3, space="PSUM") as psq_p,
                tc.tile_pool(name="psvt", bufs=2, space="PSUM") as psvt_p,
            ):
                # wq in 8 chunks of 4 ko so the first matmuls gate on ~0.8MB,
                # issue-interleaved with the first hid tiles
                NWG = 8
                KOG = KO // NWG
                w_grp = [
                    wpool.tile([128, KOG, (HL + 2) * D], BF16, name=f"w{g}")
                    for g in range(NWG)
                ]
                nc.sync.dma_start(out=w_grp[0][:], in_=wq[:, 0:KOG, :])
                qw_sb = miscA.tile([D, 1], F32)
                kw_sb = miscA.tile([D, 1], F32)
                eps_sb = miscA.tile([128, 1], F32)
                nc.sync.dma_start(out=qw_sb[:], in_=qwv[:])
                nc.sync.dma_start(out=kw_sb[:], in_=kwv[:])
                nc.vector.memset(eps_sb[:], EPS)

                pending_post = []

                def flush_post(keep=0):
                    while len(pending_post) > keep:
                        pending_post.pop(0)()

                for tt in range(NT):
                    tsl = slice(tt * 128, (tt + 1) * 128)
                    hid_t = hidp.tile([128, KO, 128], BF16, tag="hid")
                    nc.sync.dma_start(out=hid_t[:], in_=hidT[:, tt])
                    if tt == 0:
                        for g in range(1, NWG):
                            nc.sync.dma_start(
                                out=w_grp[g][:], in_=wq[:, g * KOG:(g + 1) * KOG, :]
                            )
                    cs_t = cspool.tile([128, 128], F32, tag="cs")
                    sn_t = cspool.tile([128, 128], F32, tag="sn")
                    nc.sync.dma_start(out=cs_t[:], in_=cs2[:, tsl])
                    nc.sync.dma_start(out=sn_t[:], in_=sn2s[:, tsl])

                    # qkv[t, c] for this t-tile: [128, 512] + [128, 256] psums
                    pq = psq_p.tile([128, 4 * D], F32, tag="pq")
                    pq2 = psq_p.tile([128, 2 * D], F32, tag="pq2")
                    for ko in range(KO):
                        lhsT = hid_t[:, ko, :]
                        wg = w_grp[ko // KOG]
                        nc.tensor.matmul(
                            pq[:], lhsT, wg[:, ko % KOG, 0:4 * D],
                            start=(ko == 0), stop=(ko == KO - 1),
                        )
                        nc.tensor.matmul(
                            pq2[:], lhsT, wg[:, ko % KOG, 4 * D:6 * D],
                            start=(ko == 0), stop=(ko == KO - 1),
                        )
                    flush_post(keep=1)

                    def make_post(tt=tt, pq=pq, pq2=pq2, tsl=tsl, cs_t=cs_t, sn_t=sn_t):
                        def _post():
                            # stage-parallel across the 5 normed heads so the
                            # ACT/DVE chains pipeline instead of serializing
                            srcs = [pq[:, m * D:(m + 1) * D] for m in range(HL)] + [pq2[:, 0:D]]
                            var, sd, rstd, ev, tp, qd, qsw = [], [], [], [], [], [], []
                            for m in range(HL + 1):
                                sqd = tmpA.tile([128, D], F32, tag="sqd", name="sqd")
                                var.append(stA.tile([128, 1], F32, tag="var", name="var"))
                                nc.scalar.activation(sqd[:], srcs[m], AF.Square, accum_out=var[m][:])
                            for m in range(HL + 1):
                                sd.append(stA.tile([128, 1], F32, tag="sd", name="sd"))
                                nc.scalar.activation(sd[m][:], var[m][:], AF.Sqrt, scale=1.0 / D, bias=eps_sb[:])
                            for m in range(HL + 1):
                                rstd.append(stA.tile([128, 1], F32, tag="rstd", name="rstd"))
                                nc.vector.reciprocal(rstd[m][:], sd[m][:])
                            for m in range(HL + 1):
                                ev.append(tmpA.tile([128, D], BF16, tag="ev", name="ev"))
                                nc.scalar.activation(ev[m][:], srcs[m], AF.Copy, scale=rstd[m][:])
                            for m in range(HL + 1):
                                tp.append(psvt_p.tile([128, 128], BF16, tag="tp", name="tp"))
                                nc.tensor.transpose(tp[m][:], ev[m][:], ident[:])
                            for m in range(HL + 1):
                                qd.append(tmpA.tile([128, D], F32, tag="qd", name="qd"))
                                nc.scalar.activation(
                                    qd[m][:], tp[m][:], AF.Copy,
                                    scale=(qw_sb[:] if m < HL else kw_sb[:]),
                                )
                            for m in range(HL + 1):
                                qsw.append(tmpA.tile([128, D], F32, tag="qsw", name="qsw"))
                                nc.vector.tensor_copy(qsw[m][0:64, :], qd[m][64:128, :])
                                nc.vector.tensor_copy(qsw[m][64:128, :], qd[m][0:64, :])
                            for m in range(HL + 1):
                                nc.vector.tensor_mul(qd[m][:], qd[m][:], cs_t[:])
                                nc.vector.tensor_mul(qsw[m][:], qsw[m][:], sn_t[:])
                                dst = qT[m][:, tsl] if m < HL else kT[:, tsl]
                                nc.vector.tensor_add(dst, qd[m][:], qsw[m][:])
                            nc.vector.tensor_copy(vnat[:, tt, :], pq2[:, D:2 * D])
                        return _post

                    pending_post.append(make_post())
                flush_post()

            # ---------------- Phase B: attention + o_proj + reduce-scatter ---
            with (
                tc.tile_pool(name="persistB", bufs=1) as pB,
                tc.tile_pool(name="exp", bufs=6) as exp_p,
                tc.tile_pool(name="stB", bufs=2) as stB,
                tc.tile_pool(name="ostg", bufs=12) as ostg_p,
                tc.tile_pool(name="psc", bufs=4, space="PSUM") as psc_p,
                tc.tile_pool(name="psav", bufs=1, space="PSUM") as psav_p,
                tc.tile_pool(name="pssum", bufs=1, space="PSUM") as pssum_p,
                tc.tile_pool(name="pso", bufs=2, space="PSUM") as pso_p,
                tc.tile_pool(name="dramB", bufs=1, space="DRAM") as dramB,
            ):
                attnT = pB.tile([128, HL, T], BF16)
                wo_sb = pB.tile([128, HL, HID], BF16)
                mask_sb = pB.tile([128, len(MASK_DELTAS), 2 * SC], BF16)
                nc.sync.dma_start(out=mask_sb[:], in_=masks[:])
                nc.gpsimd.dma_start(out=wo_sb[:], in_=wo[:])

                # one partial tile per RS group: a shared tile would make every
                # o_proj DMA write for later rows depend on the in-flight RS
                # read of earlier rows (observed as 10us PE stalls)
                partial = [
                    dramB.tile([r1 - r0, HID], BF16, name=f"partial{g}")
                    for g, (r0, r1) in enumerate(RS_GROUPS)
                ]
                rs_out = [
                    dramB.tile([(r1 - r0) // N_CORES, HID], BF16, name=f"rsout{g}")
                    for g, (r0, r1) in enumerate(RS_GROUPS)
                ]

                def rs_group_of(trow):
                    for g, (r0, r1) in enumerate(RS_GROUPS):
                        if r0 <= trow * 128 < r1:
                            return g, r0
                    raise AssertionError(trow)

                # o_proj emitted as small PE groups, interleaved into the next
                # sub-slab's attention loop as filler so the PE stream stays
                # dense while the norm chain drains on DVE
                oproj_q = []

                def emit_oproj_group():
                    if oproj_q:
                        oproj_q.pop(0)()

                def queue_oproj(sc):
                    def make_group(trow, ec, rs_group):
                        def _g():
                            pso = pso_p.tile([128, ECH], F32, tag="pso", name="pso")
                            for h in range(HL):
                                nc.tensor.matmul(
                                    pso[:],
                                    attnT[:, h, trow * 128:(trow + 1) * 128],
                                    wo_sb[:, h, ec * ECH:(ec + 1) * ECH],
                                    start=(h == 0),
                                    stop=(h == HL - 1),
                                )
                            ost = ostg_p.tile([128, ECH], BF16, tag="ost", name="ost")
                            nc.scalar.activation(ost[:], pso[:], AF.Copy)
                            g, gr0 = rs_group_of(trow)
                            lrow = trow * 128 - gr0
                            nc.sync.dma_start(
                                out=partial[g][lrow:lrow + 128,
                                               ec * ECH:(ec + 1) * ECH],
                                in_=ost[:],
                            )
                            if rs_group is not None:
                                r0, r1 = RS_GROUPS[rs_group]
                                rows_c = (r1 - r0) // N_CORES
                                nc.gpsimd.collective_compute(
                                    "ReduceScatter",
                                    mybir.AluOpType.add,
                                    replica_groups=[list(range(N_CORES))],
                                    ins=[partial[rs_group][:]],
                                    outs=[rs_out[rs_group][:]],
                                )
                                # same gpsimd queue as the RS, so this wait
                                # can't poison other engines
                                o0 = r0 // N_CORES
                                nc.gpsimd.dma_start(
                                    out=out_p[o0:o0 + rows_c, :],
                                    in_=rs_out[rs_group][:],
                                )
                        return _g

                    rows = [sc * 2, sc * 2 + 1]
                    n = len(rows) * N_ECH
                    i = 0
                    for trow in rows:
                        for ec in range(N_ECH):
                            i += 1
                            rsg = RS_AFTER_SC.get(sc) if i == n else None
                            oproj_q.append(make_group(trow, ec, rsg))

                # flattened (sc, hp, si) stream with lookahead across (sc, hp)
                # boundaries so the PE never drains on the exp/norm chains
                steps = []
                for sc in range(N_SC):
                    q0 = sc * SC
                    si_lo = max(0, (q0 - (WINDOW - 1)) // 128)
                    sis = list(range(si_lo, 2 * sc + 2))
                    for hp in range(0, HL, 2):
                        for si in sis:
                            steps.append((sc, hp, si, sis[0], sis[-1]))

                exs = {}
                acc = {}

                def emit_scores(step):
                    sc, hp, si, _, _ = step
                    q0 = sc * SC
                    qsl = slice(q0, q0 + SC)
                    delta = q0 - si * 128
                    # both heads of the pair share one psum/exp tile
                    # ([:, 0:SC] = head hp, [:, SC:2SC] = head hp+1)
                    psc = psc_p.tile([128, 2 * SC], F32, tag="sc")
                    for j in range(2):
                        nc.tensor.matmul(
                            psc[:, j * SC:(j + 1) * SC],
                            kT[:, si * 128:(si + 1) * 128],
                            qT[hp + j][:, qsl], start=True, stop=True,
                        )
                    ex = exp_p.tile([128, 2 * SC], BF16, tag="ex")
                    nc.scalar.activation(ex[:], psc[:], AF.Exp)
                    if delta in MASK_IDX:
                        nc.vector.tensor_mul(
                            ex[:], ex[:], mask_sb[:, MASK_IDX[delta], :]
                        )
                    exs[(sc, hp, si)] = ex

                def emit_consume(step):
                    sc, hp, si, si0, si_last = step
                    q0 = sc * SC
                    qsl = slice(q0, q0 + SC)
                    first = si == si0
                    last = si == si_last
                    if first:
                        # GQA: one AV / one sum matmul serves both heads
                        acc[(sc, hp)] = (
                            psav_p.tile([128, 2 * SC], F32, tag="av", name="av"),
                            pssum_p.tile([128, 2 * SC], F32, tag="sum", name="sum"),
                        )
                    avs2, sums2 = acc[(sc, hp)]
                    ex = exs.pop((sc, hp, si))
                    nc.tensor.matmul(sums2[:], onc[:], ex[:], start=first, stop=last)
                    nc.tensor.matmul(avs2[:], vnat[:, si, :], ex[:], start=first, stop=last)
                    if last:
                        avs2, sums2 = acc.pop((sc, hp))
                        rws2 = stB.tile([128, 2 * SC], F32, tag="rws", name="rws")
                        nc.vector.reciprocal(rws2[:], sums2[:])
                        for j in range(2):
                            nc.vector.tensor_mul(
                                attnT[:, hp + j, qsl],
                                avs2[:, j * SC:(j + 1) * SC],
                                rws2[:, j * SC:(j + 1) * SC],
                            )
                        if hp == 2:
                            queue_oproj(sc)

                LOOK = 2
                for idx, step in enumerate(steps):
                    emit_scores(step)
                    if idx >= LOOK:
                        emit_consume(steps[idx - LOOK])
                        emit_oproj_group()
                        emit_oproj_group()
                for idx in range(len(steps) - LOOK, len(steps)):
                    emit_consume(steps[idx])
                    emit_oproj_group()
                    emit_oproj_group()
                while oproj_q:
                    emit_oproj_group()

    nc.finalize()
    return nc


_NC_CACHE = None


def _get_nc():
    global _NC_CACHE
    if _NC_CACHE is None:
        _NC_CACHE = _build()
    return _NC_CACHE


def _host_inputs(positions, hidden_states, w_qkv, q_norm_w, k_norm_w, w_o):
    positions = np.asarray(positions)
    hidden_states = np.asarray(hidden_states, dtype=np.float32)
    w_qkv = np.asarray(w_qkv, dtype=np.float32)
    q_norm_w = np.asarray(q_norm_w, dtype=np.float32)
    k_norm_w = np.asarray(k_norm_w, dtype=np.float32)
    w_o = np.asarray(w_o, dtype=np.float32)

    # [ki, tt, ko, j]
    hidT4 = np.ascontiguousarray(
        hidden_states.T.reshape(KO, 128, NT, 128).transpose(1, 2, 0, 3)
    ).astype(ml_dtypes.bfloat16)

    half = D // 2
    inv_freq = 1.0 / (THETA ** (np.arange(half, dtype=np.float32) / half))
    ang = positions.astype(np.float32)[:, None] * inv_freq[None, :]  # [T, 64]
    cos = np.cos(ang).T.astype(np.float32)   # [64, T]
    sin = np.sin(ang).T.astype(np.float32)
    cs2 = np.concatenate([cos, cos], axis=0)          # [128, T]
    sn2s = np.concatenate([-sin, sin], axis=0)        # [128, T]

    # multiplicative 0/1 masks on the exp output, duplicated for the two
    # heads sharing one [128, 2*SC] tile
    mk = np.zeros((len(MASK_DELTAS), 128, SC), np.float32)
    ss = np.arange(128)[:, None]
    ttv = np.arange(SC)[None, :]
    for i, dlt in enumerate(MASK_DELTAS):
        diff = dlt + ttv - ss
        mk[i] = ((diff >= 0) & (diff < WINDOW)).astype(np.float32)
    mk = np.concatenate([mk, mk], axis=2)  # [nd, 128, 2*SC]
    mk = np.ascontiguousarray(mk.transpose(1, 0, 2)).astype(ml_dtypes.bfloat16)

    qwv = (q_norm_w * (D ** -0.5)).reshape(D, 1).astype(np.float32)
    kwv = k_norm_w.reshape(D, 1).astype(np.float32)
    onc = np.ones((128, 128), ml_dtypes.bfloat16)

    in_maps = []
    for c in range(N_CORES):
        wq_c = np.concatenate(
            [
                w_qkv[:, c * HL * D:(c + 1) * HL * D],
                w_qkv[:, H * D + c * D:H * D + (c + 1) * D],
                w_qkv[:, (H + HKV) * D + c * D:(H + HKV) * D + (c + 1) * D],
            ],
            axis=1,
        )
        wq_c = np.ascontiguousarray(wq_c.reshape(KO, 128, (HL + 2) * D).transpose(1, 0, 2)).astype(ml_dtypes.bfloat16)
        wo_c = np.ascontiguousarray(
            w_o[c * HL * D:(c + 1) * HL * D, :].reshape(HL, 128, HID).transpose(1, 0, 2)
        ).astype(ml_dtypes.bfloat16)
        in_maps.append(
            {
                "hidT": hidT4,
                "wq": wq_c,
                "wo": wo_c,
                "cs2": cs2,
                "sn2s": sn2s,
                "masks": mk,
                "qwv": qwv,
                "kwv": kwv,
                "onc": onc,
            }
        )
    return in_maps


def _assemble(results):
    out = np.empty((T, HID), np.float32)
    for c in range(N_CORES):
        r = np.asarray(results[c]["out"], dtype=np.float32)  # [T//8, HID]
        for r0, r1 in RS_GROUPS:
            rows_c = (r1 - r0) // N_CORES
            o0 = r0 // N_CORES
            out[r0 + c * rows_c: r0 + (c + 1) * rows_c] = r[o0:o0 + rows_c]
    return out


def run_spmd(in_maps, trace=False, **kw):
    nc = _get_nc()
    return run_bass_kernel_spmd(nc, in_maps, list(range(N_CORES)), trace=trace, **kw)


def kernel(positions, hidden_states, w_qkv, q_norm_w, k_norm_w, w_o):
    in_maps = _host_inputs(positions, hidden_states, w_qkv, q_norm_w, k_norm_w, w_o)
    last_err = None
    for _ in range(3):
        try:
            res = run_spmd(in_maps)
            return _assemble(res.results)
        except Exception as e:  # rare transient NRT_EXEC_UNIT_UNRECOVERABLE
            last_err = e
    raise last_err
